# revision 1
# baseline (speedup 1.0000x reference)
"""Trainium2 Bass kernel for nn_CrowdsClassificationCModel.

Computes, for B x (C,C,R) annotator confusion tensors:
    logits = einsum('bf,fkr->bkr', x, W).reshape(B,C,C,R) + b
    M      = softmax(logits, axis=2)           # over predicted-class d
    out    = einsum('bc,bcdr->bdr', p, M)      # (B, C, R)

Sharding: pure data-parallel over B across 8 NeuronCores; W/b replicated.

Per-core dataflow (Bs = 2048 batch rows; k = c*512 + d*64 + r, 32 chunks of 128):
  - PE:  logits chunks (128k x 512b) = W2_chunk.T @ xT      (bf16 inputs, f32 PSUM)
  - ACT: E0 = exp(logits)  PSUM->SBUF bf16, 1024-wide instructions
  - PE:  S_dup (128 x 512b) = sum_d exp(b)*E0 via masked matmuls
         (the mask carries exp(bias); softmax bias add becomes a multiply)
  - DVE: sinv = approx 1/S;  GpSimd: qd = sinv * p_bcast
  - DVE: Eq = E0 * qd  (plain bf16 TT, 2x mode; in place)
  - PE:  c-sum with bias: psum(128k' x 512b) += diag(exp(b)).T @ Eq_chunk,
         accumulated over c (k-major output; host transposes after gather)
  - DVE/ACT: PSUM->SBUF copy;  DMA out (contiguous 2KB rows)
"""

import numpy as np
import ml_dtypes

BF = ml_dtypes.bfloat16
NCORES = 8
B_FULL = 16384
BS = B_FULL // NCORES   # 2048 rows per core
F = 128
C = 8
R = 64
K = C * C * R           # 4096
NCHUNK = K // 128       # 32 k-chunks
NB = 4                  # b-chunks of 512 per core
BCH = BS // NB          # 512

_CACHE = {}


def _build_nc():
    import concourse.bass as bass
    import concourse.bacc as bacc
    import concourse.tile as tile
    from concourse import mybir
    from contextlib import ExitStack

    f32 = mybir.dt.float32
    bf16 = mybir.dt.bfloat16
    Exp = mybir.ActivationFunctionType.Exp
    MUL = mybir.AluOpType.mult

    nc = bacc.Bacc()
    xT = nc.declare_dram_parameter("xT", [128, BS], bf16, isOutput=False)
    W2 = nc.declare_dram_parameter("W2", [128, K], bf16, isOutput=False)
    pT = nc.declare_dram_parameter("pT", [C, BS], f32, isOutput=False)
    msk = nc.declare_dram_parameter("msk", [128, K], bf16, isOutput=False)
    dgb = nc.declare_dram_parameter("dgb", [128, K], bf16, isOutput=False)
    # k-major output: row k' = d*64+r, col b; host transposes after gather
    out = nc.declare_dram_parameter("out", [C * R, BS], f32, isOutput=True)

    with ExitStack() as ctx:
        tc = ctx.enter_context(tile.TileContext(nc))
        const = ctx.enter_context(tc.tile_pool(name="const", bufs=1))
        epool = ctx.enter_context(tc.tile_pool(name="e", bufs=2))
        sm = ctx.enter_context(tc.tile_pool(name="sm", bufs=3))
        pbp = ctx.enter_context(tc.tile_pool(name="pbp", bufs=2))
        plg = ctx.enter_context(tc.tile_pool(name="plg", bufs=2, space="PSUM"))
        ps = ctx.enter_context(tc.tile_pool(name="ps", bufs=2, space="PSUM"))
        pot = ctx.enter_context(tc.tile_pool(name="pot", bufs=2, space="PSUM"))

        # const loads, split so the first chunks land fast: W2 pieces on SP
        # (PE needs chunk 0 first), xT pieces in parallel on the ACT queue,
        # masks follow on SP before first use
        W2s = const.tile([128, K], bf16)
        xTs = const.tile([128, BS], bf16)
        nc.sync.dma_start(out=W2s[:, 0:512], in_=W2[:, 0:512])
        nc.scalar.dma_start(out=xTs[:, 0:BCH], in_=xT[:, 0:BCH])
        for i in range(1, 8):
            nc.sync.dma_start(out=W2s[:, i * 512:(i + 1) * 512],
                              in_=W2[:, i * 512:(i + 1) * 512])
        for i in range(1, NB):
            nc.scalar.dma_start(out=xTs[:, i * BCH:(i + 1) * BCH],
                                in_=xT[:, i * BCH:(i + 1) * BCH])
        msks = const.tile([128, K], bf16)
        nc.sync.dma_start(out=msks, in_=msk[:, :])
        dgbs = const.tile([128, K], bf16)
        nc.sync.dma_start(out=dgbs, in_=dgb[:, :])

        def emit_out_group(etiles, bc, g):
            """c-sum + bias for output k-chunk g of batch-chunk bc.

            out_kb[k', b] = sum_c expb[k'] * Eq[c, k', b] — diag(expb) is the
            stationary operand, Eq streams 512 columns per matmul.
            """
            pog = pot.tile([128, BCH], f32, tag="ot")
            for c in range(C):
                j = c * 4 + g
                src = etiles[(c, g // 2)][:, (g % 2) * 512:(g % 2 + 1) * 512]
                nc.tensor.matmul(
                    pog, lhsT=dgbs[:, j * 128:(j + 1) * 128], rhs=src,
                    start=(c == 0), stop=(c == C - 1),
                )
            osb = sm.tile([128, BCH], f32, tag="osb")
            if g % 2 == 0:
                nc.vector.tensor_copy(out=osb, in_=pog)
            else:
                nc.scalar.activation(
                    out=osb, in_=pog, func=mybir.ActivationFunctionType.Copy,
                )
            nc.sync.dma_start(
                out=out[g * 128:(g + 1) * 128, bc * BCH:(bc + 1) * BCH],
                in_=osb,
            )

        def load_pb(bc):
            # batched broadcast load of p for all 8 classes of b-chunk bc:
            # pb_all[pp, c*512 + i] = p[bc*512 + i, c]  (same for all pp)
            t = pbp.tile([128, C, BCH], f32, tag="pb")
            nc.gpsimd.dma_start(
                out=t,
                in_=bass.AP(tensor=pT.handle if hasattr(pT, "handle") else pT,
                            offset=bc * BCH,
                            ap=[[0, 128], [BS, C], [1, BCH]]),
            )
            return t

        pb_next = load_pb(0)
        prev = None  # (bc, etiles) awaiting output emission
        for bc in range(NB):
            bsl = slice(bc * BCH, (bc + 1) * BCH)
            pb_all = pb_next
            etiles = {}
            for c in range(C):
                for h in range(2):
                    lg = plg.tile([128, 1024], f32, tag="lg")
                    for d2 in range(2):
                        j = c * 4 + h * 2 + d2
                        nc.tensor.matmul(
                            lg[:, d2 * 512:(d2 + 1) * 512],
                            lhsT=W2s[:, j * 128:(j + 1) * 128],
                            rhs=xTs[:, bsl],
                            start=True, stop=True,
                        )
                    E = epool.tile([128, 1024], bf16, tag=f"e{c}_{h}")
                    nc.scalar.activation(out=E, in_=lg, func=Exp)
                    etiles[(c, h)] = E
                # weighted d-sum -> S duplicated over both 64-partition halves
                sps = ps.tile([128, BCH], f32, tag="s")
                for dj in range(4):
                    j = c * 4 + dj
                    nc.tensor.matmul(
                        sps,
                        lhsT=msks[:, j * 128:(j + 1) * 128],
                        rhs=etiles[(c, dj // 2)][:, (dj % 2) * 512:(dj % 2 + 1) * 512],
                        start=(dj == 0), stop=(dj == 3),
                    )
                sinv = sm.tile([128, BCH], f32, tag="sinv")
                nc.vector.reciprocal_approx_fast(out=sinv, in_=sps)
                qd = sm.tile([128, BCH], bf16, tag="qd")
                nc.gpsimd.tensor_tensor(out=qd, in0=sinv, in1=pb_all[:, c, :], op=MUL)
                for dj in range(4):
                    sl = etiles[(c, dj // 2)][:, (dj % 2) * 512:(dj % 2 + 1) * 512]
                    nc.vector.tensor_tensor(out=sl, in0=sl, in1=qd, op=MUL)
                # interleave previous chunk's output phase to keep ACT/PE dense
                if prev is not None and c % 2 == 1:
                    emit_out_group(prev[1], prev[0], c // 2)
                # prefetch next b-chunk's p broadcast mid-chunk
                if c == 3 and bc + 1 < NB:
                    pb_next = load_pb(bc + 1)
            prev = (bc, etiles)
        for g in range(4):
            emit_out_group(prev[1], prev[0], g)
    nc.compile()
    return nc


def _host_prep(x, p, W, b):
    W2 = np.ascontiguousarray(W.reshape(F, K).astype(BF))
    bflat = b.reshape(K).astype(np.float32)
    expb_bf = np.exp(bflat).astype(BF)            # single rounding, reused everywhere
    eye64 = (np.arange(128)[:, None] % 64) == (np.arange(128)[None, :] % 64)
    msk = np.zeros((128, K), dtype=BF)
    dgb = np.zeros((128, K), dtype=BF)
    for j in range(NCHUNK):
        col = expb_bf[j * 128:(j + 1) * 128].astype(np.float32)
        msk[:, j * 128:(j + 1) * 128] = np.where(eye64, col[:, None], 0.0).astype(BF)
        dgb[:, j * 128:(j + 1) * 128] = np.diag(col).astype(BF)
    xT_all = np.ascontiguousarray(x.T.astype(BF))       # (128, B)
    pT_all = np.ascontiguousarray(p.T.astype(np.float32))  # (8, B)
    in_maps = []
    for ci in range(NCORES):
        sl = slice(ci * BS, (ci + 1) * BS)
        in_maps.append({
            "xT": np.ascontiguousarray(xT_all[:, sl]),
            "W2": W2,
            "pT": np.ascontiguousarray(pT_all[:, sl]),
            "msk": msk,
            "dgb": dgb,
        })
    return in_maps


def kernel(x, p, W, b):
    from concourse.bass_utils import run_bass_kernel_spmd

    if "nc" not in _CACHE:
        _CACHE["nc"] = _build_nc()
    nc = _CACHE["nc"]
    in_maps = _host_prep(np.asarray(x), np.asarray(p), np.asarray(W), np.asarray(b))
    res = run_bass_kernel_spmd(nc, in_maps, list(range(NCORES)))
    outs = [np.asarray(res.results[i]["out"]) for i in range(NCORES)]  # (C*R, BS)
    full = np.concatenate(outs, axis=1)              # (C*R, B)
    full = np.ascontiguousarray(full.T)              # (B, C*R)
    return full.reshape(B_FULL, C, R).astype(np.float32)



# revision 9
# speedup vs baseline: 1.1219x; 1.1219x over previous
"""Trainium2 Bass kernel for nn_CrowdsClassificationCModel.

Computes, for B x (C,C,R) annotator confusion tensors:
    logits = einsum('bf,fkr->bkr', x, W).reshape(B,C,C,R) + b
    M      = softmax(logits, axis=2)           # over predicted-class d
    out    = einsum('bc,bcdr->bdr', p, M)      # (B, C, R)

Sharding: pure data-parallel over B across 8 NeuronCores; W/b replicated.

Per-core dataflow (Bs = 2048 batch rows; k = c*512 + d*64 + r, 32 chunks of 128):
  - PE:  logits chunks (128k x 512b) via fp8e4m3 DoubleRow matmuls
         (f=128 contraction packed as 2 k-tiles of 64 partitions; x,W
         pre-scaled by 16 on host, descaled by exp's scale=1/256)
  - ACT: E0 = exp(logits/256)  PSUM->SBUF bf16, 1024-wide instructions
  - PE:  S_dup (128 x 512b) = sum_d exp(b)*E0 via masked matmuls
         (the mask carries exp(bias); softmax bias add becomes a multiply)
  - Pool: qd = p / S  (single gpsimd divide, reads S straight from PSUM)
  - DVE: Eq = E0 * qd  (bf16 TT, 2x mode, in place; 1024-wide w/ bcast)
  - PE:  c-sum with bias: psum(128k' x 512b) += diag(exp(b)).T @ Eq_chunk,
         accumulated over c (k-major output; host transposes after gather)
  - Pool: PSUM->SBUF copy;  DMA out (contiguous 2KB rows)
"""

import numpy as np
import ml_dtypes

BF = ml_dtypes.bfloat16
F8 = ml_dtypes.float8_e4m3
NCORES = 8
B_FULL = 16384
BS = B_FULL // NCORES   # 2048 rows per core
F = 128
C = 8
R = 64
K = C * C * R           # 4096
NCHUNK = K // 128       # 32 k-chunks
NB = 4                  # b-chunks of 512 per core
BCH = BS // NB          # 512
FP8_SCALE = 16.0        # x and W each scaled by 16 -> exp descales by 1/256

_CACHE = {}


def _build_nc():
    import concourse.bass as bass
    import concourse.bacc as bacc
    import concourse.tile as tile
    from concourse import mybir
    from contextlib import ExitStack

    f32 = mybir.dt.float32
    bf16 = mybir.dt.bfloat16
    fp8 = mybir.dt.float8e4
    Exp = mybir.ActivationFunctionType.Exp
    MUL = mybir.AluOpType.mult
    DIV = mybir.AluOpType.divide
    DR = mybir.MatmulPerfMode.DoubleRow

    nc = bacc.Bacc()
    x8 = nc.declare_dram_parameter("x8", [64, 2 * BS], fp8, isOutput=False)
    W8 = nc.declare_dram_parameter("W8", [64, 2 * K], fp8, isOutput=False)
    pT = nc.declare_dram_parameter("pT", [C, BS], bf16, isOutput=False)
    msk = nc.declare_dram_parameter("msk", [128, K], bf16, isOutput=False)
    dgb = nc.declare_dram_parameter("dgb", [128, K], bf16, isOutput=False)
    # k-major output: row k' = d*64+r, col b; host transposes after gather
    out = nc.declare_dram_parameter("out", [C * R, BS], f32, isOutput=True)

    with ExitStack() as ctx:
        tc = ctx.enter_context(tile.TileContext(nc))
        const = ctx.enter_context(tc.tile_pool(name="const", bufs=1))
        epool = ctx.enter_context(tc.tile_pool(name="e", bufs=2))
        sm = ctx.enter_context(tc.tile_pool(name="sm", bufs=3))
        pbp = ctx.enter_context(tc.tile_pool(name="pbp", bufs=2))
        plg = ctx.enter_context(tc.tile_pool(name="plg", bufs=2, space="PSUM"))
        ps = ctx.enter_context(tc.tile_pool(name="ps", bufs=2, space="PSUM"))
        pot = ctx.enter_context(tc.tile_pool(name="pot", bufs=2, space="PSUM"))

        # const loads: W8 pieces first on SP (PE needs chunk 0 first), x8 on
        # the DVE queue, masks follow on SP before first use
        W8s = const.tile([64, 2, K], fp8)
        x8s = const.tile([64, 2, BS], fp8)
        nc.sync.dma_start(out=W8s[:, :, 0:512], in_=W8[:, 0:1024])
        nc.gpsimd.dma_start(out=x8s, in_=x8[:, :])
        for i in range(1, 8):
            nc.sync.dma_start(out=W8s[:, :, i * 512:(i + 1) * 512],
                              in_=W8[:, i * 1024:(i + 1) * 1024])
        msks = const.tile([128, K], bf16)
        nc.sync.dma_start(out=msks, in_=msk[:, :])
        dgbs = const.tile([128, K], bf16)
        nc.sync.dma_start(out=dgbs, in_=dgb[:, :])

        def emit_out_group(etiles, bc, g):
            """c-sum + bias for output k-chunk g of batch-chunk bc.

            out_kb[k', b] = sum_c expb[k'] * Eq[c, k', b] — diag(expb) is the
            stationary operand, Eq streams 512 columns per matmul.
            """
            pog = pot.tile([128, BCH], f32, tag="ot")
            for c in range(C):
                j = c * 4 + g
                src = etiles[(c, g // 2)][:, (g % 2) * 512:(g % 2 + 1) * 512]
                nc.tensor.matmul(
                    pog, lhsT=dgbs[:, j * 128:(j + 1) * 128], rhs=src,
                    start=(c == 0), stop=(c == C - 1),
                )
            osb = sm.tile([128, BCH], f32, tag="osb")
            nc.vector.tensor_copy(out=osb, in_=pog)
            nc.gpsimd.dma_start(
                out=out[g * 128:(g + 1) * 128, bc * BCH:(bc + 1) * BCH],
                in_=osb,
            )

        def load_pb(bc):
            # batched broadcast load of p for all 8 classes of b-chunk bc:
            # pb_all[pp, c*512 + i] = p[bc*512 + i, c]  (same for all pp)
            t = pbp.tile([128, C, BCH], bf16, tag="pb")
            nc.sync.dma_start(
                out=t,
                in_=bass.AP(tensor=pT.handle if hasattr(pT, "handle") else pT,
                            offset=bc * BCH,
                            ap=[[0, 128], [BS, C], [1, BCH]]),
            )
            return t

        pb_next = load_pb(0)
        prev = None  # (bc, etiles) awaiting output emission
        for bc in range(NB):
            bsl = slice(bc * BCH, (bc + 1) * BCH)
            pb_all = pb_next
            etiles = {}
            for c in range(C):
                for h in range(2):
                    lg = plg.tile([128, 1024], f32, tag="lg")
                    for d2 in range(2):
                        j = c * 4 + h * 2 + d2
                        nc.tensor.matmul(
                            lg[:, d2 * 512:(d2 + 1) * 512],
                            lhsT=W8s[:, :, j * 128:(j + 1) * 128],
                            rhs=x8s[:, :, bsl],
                            start=True, stop=True,
                            perf_mode=DR,
                        )
                    E = epool.tile([128, 1024], bf16, tag=f"e{c}_{h}")
                    nc.scalar.activation(out=E, in_=lg, func=Exp,
                                         scale=1.0 / (FP8_SCALE * FP8_SCALE))
                    etiles[(c, h)] = E
                # weighted d-sum -> S duplicated over both 64-partition halves
                sps = ps.tile([128, BCH], f32, tag="s")
                for dj in range(4):
                    j = c * 4 + dj
                    nc.tensor.matmul(
                        sps,
                        lhsT=msks[:, j * 128:(j + 1) * 128],
                        rhs=etiles[(c, dj // 2)][:, (dj % 2) * 512:(dj % 2 + 1) * 512],
                        start=(dj == 0), stop=(dj == 3),
                    )
                # qd = p / S: reciprocal on DVE (reads PSUM), multiply on Pool
                sinv = sm.tile([128, BCH], f32, tag="sinv")
                nc.vector.reciprocal_approx_fast(out=sinv, in_=sps)
                qd = sm.tile([128, BCH], bf16, tag="qd")
                nc.gpsimd.tensor_tensor(out=qd, in0=sinv, in1=pb_all[:, c, :], op=MUL)
                # Eq = E0 * qd in place, 1024-wide with qd broadcast over halves
                # h=0 on DVE (2x bf16), h=1 on gpsimd (Pool has slack)
                for h in range(2):
                    et = etiles[(c, h)]
                    qd_b = bass.AP(tensor=qd.tensor, offset=qd.offset,
                                   ap=[list(qd.ap[0]), [0, 2], [1, BCH]])
                    eng = nc.vector if h == 0 else nc.gpsimd
                    eng.tensor_tensor(
                        out=et.rearrange("p (t n) -> p t n", t=2),
                        in0=et.rearrange("p (t n) -> p t n", t=2),
                        in1=qd_b, op=MUL)
                # interleave previous chunk's output phase to keep ACT/PE dense
                if prev is not None and c % 2 == 1:
                    emit_out_group(prev[1], prev[0], c // 2)
                # prefetch next b-chunk's p broadcast mid-chunk
                if c == 3 and bc + 1 < NB:
                    pb_next = load_pb(bc + 1)
            prev = (bc, etiles)
        for g in range(4):
            emit_out_group(prev[1], prev[0], g)
    nc.compile()
    return nc


def _host_prep(x, p, W, b):
    W2 = W.reshape(F, K)
    bflat = b.reshape(K).astype(np.float32)
    expb_bf = np.exp(bflat).astype(BF)            # single rounding, reused everywhere
    eye64 = (np.arange(128)[:, None] % 64) == (np.arange(128)[None, :] % 64)
    msk = np.zeros((128, K), dtype=BF)
    dgb = np.zeros((128, K), dtype=BF)
    for j in range(NCHUNK):
        col = expb_bf[j * 128:(j + 1) * 128].astype(np.float32)
        msk[:, j * 128:(j + 1) * 128] = np.where(eye64, col[:, None], 0.0).astype(BF)
        dgb[:, j * 128:(j + 1) * 128] = np.diag(col).astype(BF)
    # fp8 DoubleRow packing: [64, 2, *] with f split into two 64-deep k-tiles
    xT_all = (x.T * FP8_SCALE).astype(F8)                 # (128, B)
    x8_all = np.ascontiguousarray(xT_all.reshape(2, 64, B_FULL).transpose(1, 0, 2))
    # pack to match the 8-piece on-chip loads: piece i holds [t=0 | t=1] for
    # k-block i*512:(i+1)*512 (dest tile iterates (t, k) per piece)
    W8v = (W2 * FP8_SCALE).astype(F8).reshape(2, 64, K).transpose(1, 0, 2)  # (64,2,K)
    W8 = np.ascontiguousarray(
        W8v.reshape(64, 2, 8, 512).transpose(0, 2, 1, 3)
    ).reshape(64, 2 * K)
    pT_all = np.ascontiguousarray(p.T.astype(BF))         # (8, B)
    in_maps = []
    for ci in range(NCORES):
        sl = slice(ci * BS, (ci + 1) * BS)
        in_maps.append({
            "x8": np.ascontiguousarray(x8_all[:, :, sl]).reshape(64, 2 * BS),
            "W8": W8,
            "pT": np.ascontiguousarray(pT_all[:, sl]),
            "msk": msk,
            "dgb": dgb,
        })
    return in_maps


def kernel(x, p, W, b):
    from concourse.bass_utils import run_bass_kernel_spmd

    if "nc" not in _CACHE:
        _CACHE["nc"] = _build_nc()
    nc = _CACHE["nc"]
    in_maps = _host_prep(np.asarray(x), np.asarray(p), np.asarray(W), np.asarray(b))
    res = run_bass_kernel_spmd(nc, in_maps, list(range(NCORES)))
    outs = [np.asarray(res.results[i]["out"]) for i in range(NCORES)]  # (C*R, BS)
    full = np.concatenate(outs, axis=1)              # (C*R, B)
    full = np.ascontiguousarray(full.T)              # (B, C*R)
    return full.reshape(B_FULL, C, R).astype(np.float32)
